# revision 30
# baseline (speedup 1.0000x reference)
"""BinaryTreeLSTM Trainium2 kernel (B=32 trees, 512 leaves, dim 1024).

Sharding: data-parallel over trees -- 4 trees per core on 8 NeuronCores,
gate weights replicated.

Per-core design:
  - Activations are kept feature-major [feat_chunk(128), tree, node_col].
  - Leaves are pre-permuted on the host by 9-bit bit-reversal, which makes
    the two children of output node j sit at columns (j, j+n) at *every*
    level -> all child reads are contiguous block slices (no strided APs).
  - Matmul operands fp16 (the 20 MiB of transposed gate weights stay
    SBUF-resident the whole kernel; their load is interleaved with the
    leaf phase), PSUM accumulation and elementwise math fp32, h AND c
    stored fp16 between levels (packed [h|c] in one DRAM scratch tensor
    so each store/load is a single batched DMA -- HWDGE descriptor issue
    at ~0.6us/DMA was half the leaf phase at the old 1-DMA-per-tensor
    granularity).
  - The leaf level skips the forget-gate matmul (child state is zero).
  - Levels down to n=64 round-trip h/c (fp16) through DRAM scratch;
    n<=32 keeps state in SBUF.  The n<=16 tail levels process all 8
    feature chunks in ONE psum tile per gate: the gate bias is pre-filled
    into psum (DVE broadcast copy), matmuls accumulate on top
    (start=False), and the activations/elementwise run once per gate over
    [128, MF*T*n] instead of once per (m, gate) -- 8x fewer tiny ops on
    the Act/DVE critical path.
"""

import sys

if "/opt/trn_rl_repo" not in sys.path:
    sys.path.insert(0, "/opt/trn_rl_repo")

import numpy as np

import concourse.bass as bass
import concourse.tile as tile
from concourse import bacc, mybir
from concourse.bass_utils import run_bass_kernel_spmd

F16 = mybir.dt.float16
F32 = mybir.dt.float32
AF = mybir.ActivationFunctionType

NCORES = 8
B = 32                  # trees total
T = B // NCORES         # trees per core
NL = 512                # leaves per tree
D = 1024                # IN_DIM == MEM
KX = D // 128           # 8 k-chunks for the leaf matmul
KH = 2 * D // 128       # 16 k-chunks for internal matmuls
MF = D // 128           # 8 feature chunks per gate
N_SBUF_TAIL = 32        # levels with <= this many nodes/tree keep c/h in SBUF
N_MERGED = 16           # levels with <= this many nodes/tree use merged-MF ops

_CACHE = {}
PHASES = []  # [(label, first_instruction_id)] recorded during build


def _mark(nc, label):
    PHASES.append((label, nc.next_id()))


def _bitrev(nbits):
    n = 1 << nbits
    p = np.zeros(n, np.int64)
    for i in range(n):
        r = 0
        for b in range(nbits):
            if i >> b & 1:
                r |= 1 << (nbits - 1 - b)
        p[i] = r
    return p


def _build_program(reps=1):
    """reps>1 wraps the compute body in a hardware For_i loop -- used only
    for timing (axon dispatch overhead is ~80 ms per launch, so the kernel
    must be repeated on-device to be measurable via wall-clock slope)."""
    nc = bacc.Bacc("TRN2", target_bir_lowering=False, debug=False,
                   num_devices=NCORES)
    xT = nc.dram_tensor("xT", [128, KX, T, NL], F16, kind="ExternalInput")
    wxT = nc.dram_tensor("wxT", [MF, 128, 3, KX, 128], F16,
                         kind="ExternalInput")
    whT = nc.dram_tensor("whT", [KH, 128, 5 * D], F16, kind="ExternalInput")
    biou = nc.dram_tensor("biou", [128, 3 * MF], F32, kind="ExternalInput")
    bf = nc.dram_tensor("bf", [128, MF], F32, kind="ExternalInput")
    outh = nc.dram_tensor("outh", [MF, 128, T], F32, kind="ExternalOutput")

    with tile.TileContext(nc) as tc:
        with tc.tile_pool(name="consts", bufs=1) as consts, \
             tc.tile_pool(name="whp", bufs=1) as whp, \
             tc.tile_pool(name="stream", bufs=1) as stream, \
             tc.tile_pool(name="evac", bufs=2) as evac, \
             tc.tile_pool(name="dram", bufs=2, space="DRAM") as dram, \
             tc.tile_pool(name="ps", bufs=1, space="PSUM") as ps:

            biou_sb = consts.tile([128, 3 * MF], F32)
            nc.sync.dma_start(out=biou_sb, in_=biou.ap())
            bf_sb = consts.tile([128, MF], F32)
            nc.sync.dma_start(out=bf_sb, in_=bf.ap())

            def bias_cols(gt):
                # [128, MF] fp32 per-gate bias (column m = feature chunk m)
                return {"i": biou_sb[:, 0:MF],
                        "o": biou_sb[:, MF:2 * MF],
                        "u": biou_sb[:, 2 * MF:3 * MF],
                        "fl": bf_sb[:, 0:MF],
                        "fr": bf_sb[:, 0:MF]}[gt]

            # resident gate weights: 16 k-chunks x 5120 cols fp16 (160 KiB/p)
            wh_sb = whp.tile([128, KH, 5 * D], F16)

            def load_wh_chunk(ci, kper):
                ks = slice(ci * kper, (ci + 1) * kper)
                nc.sync.dma_start(
                    out=wh_sb[:, ks],
                    in_=whT.ap()[ks].rearrange("k p c -> p k c"))

            def load_h_tile(dst, src, kh):
                # split into 2-k-slice pieces: slice-level dependency tracking
                # lets the first matmuls start as soon as piece 0 lands
                for k0 in range(0, kh, 2):
                    nc.sync.dma_start(out=dst[:, k0:k0 + 2],
                                      in_=src[:, k0:k0 + 2])

            def per_m_outputs(psl, m, cl_ap, cr_ap, hc_dst, ht_dst, ct_dst,
                              leaf, root=False):
                """Per-m-chunk gate postprocessing (big levels, free dim
                T*ncc=512).  hc_dst: DRAM AP [128, 2, T, ncc] (h slot 0,
                c slot 1) or None for SBUF mode (ht_dst/ct_dst slices).
                DVE may read at most one PSUM operand per instruction, so
                i and o evacuate to SBUF via their activations while
                u/fl/fr stay in PSUM.
                """
                shape = list(psl["i"].shape)
                if root:
                    hc_dst = None
                    ct_dst = evac.tile(shape, F32, tag="rt", bufs=2,
                                       name="rt")
                    ht_dst = None
                i_sb = evac.tile(shape, F32, tag="gsb", bufs=2, name="i_sb")
                nc.scalar.activation(out=i_sb, in_=psl["i"], func=AF.Sigmoid,
                                     bias=biou_sb[:, m:m + 1], scale=1.0)
                nc.scalar.activation(out=psl["u"], in_=psl["u"], func=AF.Tanh,
                                     bias=biou_sb[:, 2 * MF + m:2 * MF + m + 1],
                                     scale=1.0)
                c_t = evac.tile(shape, F32, tag="c", bufs=1, name="c_t")
                nc.vector.tensor_mul(c_t, i_sb, psl["u"])
                if hc_dst is not None:
                    hc_t = evac.tile([128, 2] + shape[1:], F16, tag="hc",
                                     bufs=1, name="hc_t")
                    c_dst = hc_t[:, 1]
                    h_dst = hc_t[:, 0]
                else:
                    c_dst, h_dst = ct_dst, ht_dst
                if cl_ap is not None:
                    nc.scalar.activation(out=psl["fl"], in_=psl["fl"],
                                         func=AF.Sigmoid,
                                         bias=bf_sb[:, m:m + 1], scale=1.0)
                    nc.vector.tensor_mul(psl["fl"], psl["fl"], cl_ap)
                    nc.vector.tensor_add(c_t, c_t, psl["fl"])
                    nc.scalar.activation(out=psl["fr"], in_=psl["fr"],
                                         func=AF.Sigmoid,
                                         bias=bf_sb[:, m:m + 1], scale=1.0)
                    nc.vector.tensor_mul(psl["fr"], psl["fr"], cr_ap)
                    nc.vector.tensor_add(c_dst, c_t, psl["fr"])
                else:
                    nc.vector.tensor_copy(c_dst, c_t)
                o_sb = evac.tile(shape, F32, tag="gsb", bufs=2, name="o_sb")
                nc.scalar.activation(out=o_sb, in_=psl["o"], func=AF.Sigmoid,
                                     bias=biou_sb[:, MF + m:MF + m + 1],
                                     scale=1.0)
                th = ps.tile(shape, F32, tag="th", bufs=2, name="th")
                nc.scalar.activation(out=th, in_=c_dst, func=AF.Tanh)
                if root:
                    h_t = evac.tile(shape, F32, tag="hroot", name="h_t")
                    nc.vector.tensor_mul(h_t, o_sb, th)
                    nc.sync.dma_start(out=outh.ap()[m], in_=h_t[:, :, 0])
                    return
                nc.vector.tensor_mul(h_dst, o_sb, th)
                if hc_dst is not None:
                    nc.sync.dma_start(out=hc_dst, in_=hc_t)

            def hc_scratch(n, name):
                # chunk-major: [m, col_chunk, h|c, part, tree, col] so every
                # DMA slice collapses to <=3 AP dims (t,c merge)
                W = min(n, 128)
                return dram.tile([MF, max(n // 128, 1), 2, 128, T, W], F16,
                                 tag="hc", name=name)

            def emit_leaf(wh_interleave=False):
                _mark(nc, "leaf")
                hcA = hc_scratch(NL, "hcA")
                ncc = 128
                for ci in range(NL // ncc):
                    cs = slice(ci * ncc, (ci + 1) * ncc)
                    # alternate between the (leaf-idle) hl/hr tags for
                    # double buffering at zero extra SBUF cost
                    x_t = stream.tile([128, KX, T, ncc], F16,
                                      tag=("hl" if ci % 2 == 0 else "hr"),
                                      name="x_t")
                    load_h_tile(x_t, xT.ap()[:, :, :, cs], KX)
                    for m in range(MF):
                        if wh_interleave and m % 2 == 1:
                            # spread the resident-weight load in 1.3 MiB
                            # pieces so x/wx never queue behind a large blob
                            load_wh_chunk(ci * 4 + m // 2, 1)
                        wxg = stream.tile([128, 3, KX, 128], F16, tag="wxc",
                                          bufs=2, name="wxg")
                        nc.sync.dma_start(out=wxg, in_=wxT.ap()[m])
                        psl = {}
                        for g, gt in enumerate(("i", "o", "u")):
                            pt = ps.tile([128, T, ncc], F32, tag=gt,
                                         bufs=(2 if gt == "o" else 1),
                                         name=f"ps_{gt}")
                            for k in range(KX):
                                nc.tensor.matmul(pt, lhsT=wxg[:, g, k],
                                                 rhs=x_t[:, k],
                                                 start=(k == 0),
                                                 stop=(k == KX - 1))
                            psl[gt] = pt
                        per_m_outputs(
                            psl, m, None, None,
                            hcA[m, ci].rearrange("s p t c -> p s t c"),
                            None, None, leaf=True)
                return hcA

            def emit_merged_level(n, ht_prev, ct_prev, root):
                """Tail level (n<=16 output nodes/tree): all MF chunks in one
                psum tile per gate; bias pre-filled, merged elementwise."""
                shape = [128, MF, T, n]
                ht_cur = ct_cur = None
                if not root:
                    ht_cur = evac.tile(shape, F16, tag="ht", bufs=2,
                                       name="ht_cur")
                    ct_cur = evac.tile(shape, F16, tag="ct", bufs=2,
                                       name="ct_cur")
                psl = {}
                for gt in ("i", "o", "u", "fl", "fr"):
                    psl[gt] = ps.tile(shape, F32, tag=gt,
                                      bufs=(2 if gt == "o" else 1),
                                      name=f"ps_{gt}")
                for g, gt in enumerate(("i", "o", "u", "fl", "fr")):
                    pt = psl[gt]
                    for m in range(MF):
                        # start=True zeroes only the written slice on HW, so
                        # per-m chains can share one bank; bias added after
                        for k in range(KH):
                            kk, lr = k % (KH // 2), k // (KH // 2)
                            nc.tensor.matmul(
                                pt[:, m],
                                lhsT=wh_sb[:, k, g * D + m * 128:
                                           g * D + (m + 1) * 128],
                                rhs=ht_prev[:, kk, :, lr * n:(lr + 1) * n],
                                start=(k == 0), stop=(k == KH - 1),
                                skip_group_check=True)
                    nc.vector.tensor_add(pt, pt,
                                         bias_cols(gt).to_broadcast(shape))
                i_sb = evac.tile(shape, F32, tag="gsb", bufs=2, name="i_sb")
                nc.scalar.activation(out=i_sb, in_=psl["i"], func=AF.Sigmoid)
                nc.scalar.activation(out=psl["u"], in_=psl["u"], func=AF.Tanh)
                c_t = evac.tile(shape, F32, tag="c", bufs=1, name="c_t")
                nc.vector.tensor_mul(c_t, i_sb, psl["u"])
                nc.scalar.activation(out=psl["fl"], in_=psl["fl"],
                                     func=AF.Sigmoid)
                nc.vector.tensor_mul(psl["fl"], psl["fl"],
                                     ct_prev[:, :, :, 0:n])
                nc.vector.tensor_add(c_t, c_t, psl["fl"])
                nc.scalar.activation(out=psl["fr"], in_=psl["fr"],
                                     func=AF.Sigmoid)
                nc.vector.tensor_mul(psl["fr"], psl["fr"],
                                     ct_prev[:, :, :, n:2 * n])
                o_sb = evac.tile(shape, F32, tag="gsb", bufs=2, name="o_sb")
                nc.scalar.activation(out=o_sb, in_=psl["o"], func=AF.Sigmoid)
                th = ps.tile(shape, F32, tag="th", bufs=2, name="th")
                if root:
                    c_f = evac.tile(shape, F32, tag="croot", name="c_f")
                    nc.vector.tensor_add(c_f, c_t, psl["fr"])
                    nc.scalar.activation(out=th, in_=c_f, func=AF.Tanh)
                    h_t = evac.tile(shape, F32, tag="hroot", name="h_t")
                    nc.vector.tensor_mul(h_t, o_sb, th)
                    nc.sync.dma_start(
                        out=outh.ap().rearrange("m p t -> p m t"),
                        in_=h_t[:, :, :, 0])
                    return None, None
                nc.vector.tensor_add(ct_cur, c_t, psl["fr"])
                nc.scalar.activation(out=th, in_=ct_cur, func=AF.Tanh)
                nc.vector.tensor_mul(ht_cur, o_sb, th)
                return ht_cur, ct_cur

            def emit_internal(hcA):
                n = NL
                ht_prev = ct_prev = None        # SBUF tail state
                while n > 1:
                    n //= 2                     # output nodes per tree
                    _mark(nc, f"level_n{n}")
                    root = n == 1
                    if n <= N_MERGED:
                        ht_prev, ct_prev = emit_merged_level(
                            n, ht_prev, ct_prev, root)
                        continue
                    sbuf_out = n <= N_SBUF_TAIL and not root
                    ht_cur = ct_cur = hcB = None
                    if root:
                        pass
                    elif sbuf_out:
                        ht_cur = evac.tile([128, MF, T, n], F16, tag="ht",
                                           bufs=2, name="ht_cur")
                        ct_cur = evac.tile([128, MF, T, n], F16, tag="ct",
                                           bufs=2, name="ct_cur")
                    else:
                        hcB = hc_scratch(n, "hcB")
                    ncc = min(n, 128)
                    small_in = 2 * n <= 128     # children fit in one chunk
                    dj = max(n // 128, 1)       # chunk distance left->right
                    for ci in range(n // ncc):
                        if small_in:
                            # children cols [0, 2n) in chunk 0: one DMA for
                            # both halves; slice left/right in SBUF
                            hl_t = stream.tile([128, KH // 2, T, 2 * n], F16,
                                               tag="hl", name="hl_t")
                            load_h_tile(hl_t,
                                        hcA[:, 0, 0].rearrange(
                                            "m p t c -> p m t c"), KH // 2)
                            hr_t = None
                        else:
                            hl_t = stream.tile([128, KH // 2, T, ncc], F16,
                                               tag="hl", name="hl_t")
                            load_h_tile(hl_t,
                                        hcA[:, ci, 0].rearrange(
                                            "m p t c -> p m t c"), KH // 2)
                            hr_t = stream.tile([128, KH // 2, T, ncc], F16,
                                               tag="hr", name="hr_t")
                            load_h_tile(hr_t,
                                        hcA[:, ci + dj, 0].rearrange(
                                            "m p t c -> p m t c"), KH // 2)
                        for m in range(MF):
                            if small_in:
                                cin = stream.tile([128, T, 2 * n], F16,
                                                  tag="wxc", bufs=2,
                                                  name="cin")
                                nc.sync.dma_start(out=cin, in_=hcA[m, 0, 1])
                                cl_ap = cin[:, :, 0:n]
                                cr_ap = cin[:, :, n:2 * n]
                            else:
                                cin = stream.tile([128, 2, T, ncc], F16,
                                                  tag="wxc", bufs=2,
                                                  name="cin")
                                nc.sync.dma_start(
                                    out=cin,
                                    in_=hcA[m, :, 1].rearrange(
                                        "(two g) p t c -> p two g t c",
                                        two=2)[:, :, ci])
                                cl_ap = cin[:, 0]
                                cr_ap = cin[:, 1]
                            psl = {}
                            for g, gt in enumerate(
                                    ("i", "o", "u", "fl", "fr")):
                                pt = ps.tile([128, T, ncc], F32, tag=gt,
                                             bufs=(2 if gt == "o" else 1),
                                             name=f"ps_{gt}")
                                for k in range(KH):
                                    kk = k % (KH // 2)
                                    if small_in:
                                        lr = k // (KH // 2)
                                        rhs = hl_t[:, kk, :,
                                                   lr * n:(lr + 1) * n]
                                    else:
                                        rhs = (hl_t[:, kk] if k < KH // 2
                                               else hr_t[:, kk])
                                    nc.tensor.matmul(
                                        pt,
                                        lhsT=wh_sb[:, k,
                                                   g * D + m * 128:
                                                   g * D + (m + 1) * 128],
                                        rhs=rhs,
                                        start=(k == 0), stop=(k == KH - 1))
                                psl[gt] = pt
                            if root:
                                per_m_outputs(psl, m, cl_ap, cr_ap,
                                              None, None, None,
                                              leaf=False, root=True)
                            elif sbuf_out:
                                per_m_outputs(psl, m, cl_ap, cr_ap,
                                              None, ht_cur[:, m], ct_cur[:, m],
                                              leaf=False)
                            else:
                                per_m_outputs(
                                    psl, m, cl_ap, cr_ap,
                                    hcB[m, ci].rearrange(
                                        "s p t c -> p s t c"),
                                    None, None, leaf=False)
                    if sbuf_out:
                        ht_prev, ct_prev = ht_cur, ct_cur
                    else:
                        hcA = hcB

            if reps == 1:
                hcA = emit_leaf(wh_interleave=True)
                emit_internal(hcA)
            else:
                for ci in range(4):
                    load_wh_chunk(ci, KH // 4)
                with tc.For_i(0, reps, 1):
                    hcA = emit_leaf()
                    emit_internal(hcA)

    nc.compile()
    return nc


def _get_nc(reps=1):
    key = f"nc{reps}"
    if key not in _CACHE:
        _CACHE[key] = _build_program(reps)
    return _CACHE[key]


def _prep_inputs(inputs, w_fioux, b_fioux, w_iouh, w_fh):
    """Host-side prep: permute/transpose/cast, build one in_map per core."""
    perm = _bitrev(9)
    # weights / biases are replicated across cores
    # wxT [m, p, g, k, c]: W_x[gate g, row m*128+c, col k*128+p]
    wxT = np.ascontiguousarray(
        w_fioux[D:].reshape(3, MF, 128, KX, 128).transpose(1, 4, 0, 3, 2)
    ).astype(np.float16)
    whT = np.ascontiguousarray(
        np.concatenate([w_iouh, w_fh], axis=0).T).astype(np.float16) \
        .reshape(KH, 128, 5 * D)
    biou = np.ascontiguousarray(
        b_fioux[D:].astype(np.float32).reshape(3 * MF, 128).T)
    bf = np.ascontiguousarray(
        b_fioux[:D].astype(np.float32).reshape(MF, 128).T)
    in_maps = []
    for c in range(NCORES):
        xc = inputs[c * T:(c + 1) * T][:, perm, :]        # [T, NL, D]
        xT = np.ascontiguousarray(
            xc.reshape(T, NL, KX, 128).transpose(3, 2, 0, 1)
        ).astype(np.float16)                              # [128, KX, T, NL]
        in_maps.append({"xT": xT, "wxT": wxT, "whT": whT,
                        "biou": biou, "bf": bf})
    return in_maps


def _assemble(results):
    out = np.zeros((B, D), np.float32)
    for c in range(NCORES):
        oh = results[c]["outh"].reshape(D, T)             # [feat, tree]
        out[c * T:(c + 1) * T] = oh.T
    return out


def kernel(inputs, w_fioux, b_fioux, w_iouh, w_fh):
    inputs = np.asarray(inputs, np.float32)
    w_fioux = np.asarray(w_fioux, np.float32)
    b_fioux = np.asarray(b_fioux, np.float32)
    w_iouh = np.asarray(w_iouh, np.float32)
    w_fh = np.asarray(w_fh, np.float32)
    nc = _get_nc()
    in_maps = _prep_inputs(inputs, w_fioux, b_fioux, w_iouh, w_fh)
    res = run_bass_kernel_spmd(nc, in_maps, core_ids=list(range(NCORES)))
    return _assemble(res.results)


# ---------------------------------------------------------------------------
# benchmarking helper (not used by the grader): builds the jitted SPMD
# callable once so repeated executions can be timed without re-lowering.
def _bench(inputs, w_fioux, b_fioux, w_iouh, w_fh, iters=20, reps=1):
    import time

    import jax
    from jax.experimental.shard_map import shard_map
    from jax.sharding import Mesh, PartitionSpec

    from concourse import bass2jax

    nc = _get_nc(reps)
    in_maps = _prep_inputs(np.asarray(inputs, np.float32),
                           np.asarray(w_fioux, np.float32),
                           np.asarray(b_fioux, np.float32),
                           np.asarray(w_iouh, np.float32),
                           np.asarray(w_fh, np.float32))
    bass2jax.install_neuronx_cc_hook()

    partition_name = (nc.partition_id_tensor.name
                      if nc.partition_id_tensor else None)
    in_names, out_names, out_avals, zero_outs = [], [], [], []
    for alloc in nc.m.functions[0].allocations:
        if not isinstance(alloc, mybir.MemoryLocationSet):
            continue
        name = alloc.memorylocations[0].name
        if alloc.kind == "ExternalInput":
            if name != partition_name:
                in_names.append(name)
        elif alloc.kind == "ExternalOutput":
            out_names.append(name)
            shape = tuple(alloc.tensor_shape)
            dtype = mybir.dt.np(alloc.dtype)
            out_avals.append(jax.core.ShapedArray(shape, dtype))
            zero_outs.append(np.zeros(shape, dtype))
    n_params = len(in_names)
    n_outs = len(out_avals)
    all_in_names = in_names + out_names + \
        ([partition_name] if partition_name else [])

    def _body(*args):
        operands = list(args)
        if partition_name is not None:
            operands.append(bass2jax.partition_id_tensor())
        outs = bass2jax._bass_exec_p.bind(
            *operands,
            out_avals=tuple(out_avals),
            in_names=tuple(all_in_names),
            out_names=tuple(out_names),
            lowering_input_output_aliases=(),
            sim_require_finite=True,
            sim_require_nnan=True,
            nc=nc,
        )
        return tuple(outs)

    devices = jax.devices()[:NCORES]
    mesh = Mesh(np.asarray(devices), ("core",))
    in_specs = (PartitionSpec("core"),) * (n_params + n_outs)
    out_specs = (PartitionSpec("core"),) * n_outs
    donate = tuple(range(n_params, n_params + n_outs))
    sharded = jax.jit(
        shard_map(_body, mesh=mesh, in_specs=in_specs, out_specs=out_specs,
                  check_rep=False),
        donate_argnums=donate, keep_unused=True)

    concat_in = [np.concatenate([in_maps[c][nm] for c in range(NCORES)],
                                axis=0)
                 for nm in in_names]
    dev_in = [jax.device_put(
        a, jax.sharding.NamedSharding(mesh, PartitionSpec("core")))
        for a in concat_in]

    def one_run():
        zeros = [jax.device_put(
            np.zeros((NCORES * z.shape[0], *z.shape[1:]), z.dtype),
            jax.sharding.NamedSharding(mesh, PartitionSpec("core")))
            for z in zero_outs]
        t0 = time.perf_counter()
        out = sharded(*dev_in, *zeros)
        jax.block_until_ready(out)
        return time.perf_counter() - t0, out

    one_run()                                    # compile warm-up
    times = []
    out = None
    for _ in range(iters):
        dt, out = one_run()
        times.append(dt)
    out_np = [np.asarray(o) for o in out]
    results = [{nm: out_np[i].reshape(NCORES, *out_avals[i].shape)[c]
                for i, nm in enumerate(out_names)}
               for c in range(NCORES)]
    return _assemble(results), times


# revision 32
# speedup vs baseline: 1.1321x; 1.1321x over previous
"""BinaryTreeLSTM Trainium2 kernel (B=32 trees, 512 leaves, dim 1024).

Sharding: data-parallel over trees -- 4 trees per core on 8 NeuronCores,
gate weights replicated.

Per-core design:
  - Activations are kept feature-major [feat_chunk(128), tree, node_col].
  - Leaves are pre-permuted on the host by 9-bit bit-reversal, which makes
    the two children of output node j sit at columns (j, j+n) at *every*
    level -> all child reads are contiguous block slices (no strided APs).
  - Matmul operands fp16 (the 20 MiB of transposed gate weights stay
    SBUF-resident the whole kernel; their load is interleaved with the
    leaf phase), PSUM accumulation and elementwise math fp32, h AND c
    stored fp16 between levels (packed [h|c] in one DRAM scratch tensor
    so each store/load is a single batched DMA -- HWDGE descriptor issue
    at ~0.6us/DMA was half the leaf phase at the old 1-DMA-per-tensor
    granularity).
  - The leaf level skips the forget-gate matmul (child state is zero).
  - Levels down to n=64 round-trip h/c (fp16) through DRAM scratch;
    n<=32 keeps state in SBUF.  The n<=16 tail levels process all 8
    feature chunks in ONE psum tile per gate: the gate bias is pre-filled
    into psum (DVE broadcast copy), matmuls accumulate on top
    (start=False), and the activations/elementwise run once per gate over
    [128, MF*T*n] instead of once per (m, gate) -- 8x fewer tiny ops on
    the Act/DVE critical path.
"""

import sys

if "/opt/trn_rl_repo" not in sys.path:
    sys.path.insert(0, "/opt/trn_rl_repo")

import numpy as np

import concourse.bass as bass
import concourse.tile as tile
from concourse import bacc, mybir
from concourse.bass_utils import run_bass_kernel_spmd

F16 = mybir.dt.float16
F32 = mybir.dt.float32
AF = mybir.ActivationFunctionType

NCORES = 8
B = 32                  # trees total
T = B // NCORES         # trees per core
NL = 512                # leaves per tree
D = 1024                # IN_DIM == MEM
KX = D // 128           # 8 k-chunks for the leaf matmul
KH = 2 * D // 128       # 16 k-chunks for internal matmuls
MF = D // 128           # 8 feature chunks per gate
N_SBUF_TAIL = 32        # levels with <= this many nodes/tree keep c/h in SBUF
N_MERGED = 16           # levels with <= this many nodes/tree use merged-MF ops
TAIL_STOP = 1           # timing experiments only: skip levels with n < this

_CACHE = {}
PHASES = []  # [(label, first_instruction_id)] recorded during build


def _mark(nc, label):
    PHASES.append((label, nc.next_id()))


def _bitrev(nbits):
    n = 1 << nbits
    p = np.zeros(n, np.int64)
    for i in range(n):
        r = 0
        for b in range(nbits):
            if i >> b & 1:
                r |= 1 << (nbits - 1 - b)
        p[i] = r
    return p


def _build_program(reps=1):
    """reps>1 wraps the compute body in a hardware For_i loop -- used only
    for timing (axon dispatch overhead is ~80 ms per launch, so the kernel
    must be repeated on-device to be measurable via wall-clock slope)."""
    nc = bacc.Bacc("TRN2", target_bir_lowering=False, debug=False,
                   num_devices=NCORES)
    xT = nc.dram_tensor("xT", [128, KX, T, NL], F16, kind="ExternalInput")
    wxT = nc.dram_tensor("wxT", [MF, 128, 3, KX, 128], F16,
                         kind="ExternalInput")
    whT = nc.dram_tensor("whT", [KH, 128, 5 * D], F16, kind="ExternalInput")
    biou = nc.dram_tensor("biou", [128, 3 * MF], F32, kind="ExternalInput")
    bf = nc.dram_tensor("bf", [128, MF], F32, kind="ExternalInput")
    outh = nc.dram_tensor("outh", [MF, 128, T], F32, kind="ExternalOutput")

    with tile.TileContext(nc) as tc:
        with tc.tile_pool(name="consts", bufs=1) as consts, \
             tc.tile_pool(name="whp", bufs=1) as whp, \
             tc.tile_pool(name="stream", bufs=1) as stream, \
             tc.tile_pool(name="evac", bufs=2) as evac, \
             tc.tile_pool(name="dram", bufs=2, space="DRAM") as dram, \
             tc.tile_pool(name="ps", bufs=1, space="PSUM") as ps:

            biou_sb = consts.tile([128, 3 * MF], F32)
            nc.sync.dma_start(out=biou_sb, in_=biou.ap())
            bf_sb = consts.tile([128, MF], F32)
            nc.sync.dma_start(out=bf_sb, in_=bf.ap())

            def bias_cols(gt):
                # [128, MF] fp32 per-gate bias (column m = feature chunk m)
                return {"i": biou_sb[:, 0:MF],
                        "o": biou_sb[:, MF:2 * MF],
                        "u": biou_sb[:, 2 * MF:3 * MF],
                        "fl": bf_sb[:, 0:MF],
                        "fr": bf_sb[:, 0:MF]}[gt]

            # resident gate weights: 16 k-chunks x 5120 cols fp16 (160 KiB/p)
            wh_sb = whp.tile([128, KH, 5 * D], F16)

            def load_wh_chunk(ci, kper):
                ks = slice(ci * kper, (ci + 1) * kper)
                nc.sync.dma_start(
                    out=wh_sb[:, ks],
                    in_=whT.ap()[ks].rearrange("k p c -> p k c"))

            def load_h_tile(dst, src, kh):
                # split into 2-k-slice pieces: slice-level dependency tracking
                # lets the first matmuls start as soon as piece 0 lands
                for k0 in range(0, kh, 2):
                    nc.sync.dma_start(out=dst[:, k0:k0 + 2],
                                      in_=src[:, k0:k0 + 2])

            def per_m_outputs(psl, m, cl_ap, cr_ap, hc_dst, ht_dst, ct_dst,
                              leaf, root=False):
                """Per-m-chunk gate postprocessing (big levels, free dim
                T*ncc=512).  hc_dst: DRAM AP [128, 2, T, ncc] (h slot 0,
                c slot 1) or None for SBUF mode (ht_dst/ct_dst slices).
                DVE may read at most one PSUM operand per instruction, so
                i and o evacuate to SBUF via their activations while
                u/fl/fr stay in PSUM.
                """
                shape = list(psl["i"].shape)
                if root:
                    hc_dst = None
                    ct_dst = evac.tile(shape, F32, tag="rt", bufs=2,
                                       name="rt")
                    ht_dst = None
                i_sb = evac.tile(shape, F32, tag="gsb", bufs=2, name="i_sb")
                nc.scalar.activation(out=i_sb, in_=psl["i"], func=AF.Sigmoid,
                                     bias=biou_sb[:, m:m + 1], scale=1.0)
                nc.scalar.activation(out=psl["u"], in_=psl["u"], func=AF.Tanh,
                                     bias=biou_sb[:, 2 * MF + m:2 * MF + m + 1],
                                     scale=1.0)
                c_t = evac.tile(shape, F32, tag="c", bufs=1, name="c_t")
                nc.vector.tensor_mul(c_t, i_sb, psl["u"])
                if hc_dst is not None:
                    hc_t = evac.tile([128, 2] + shape[1:], F16, tag="hc",
                                     bufs=1, name="hc_t")
                    c_dst = hc_t[:, 1]
                    h_dst = hc_t[:, 0]
                else:
                    c_dst, h_dst = ct_dst, ht_dst
                if cl_ap is not None:
                    nc.scalar.activation(out=psl["fl"], in_=psl["fl"],
                                         func=AF.Sigmoid,
                                         bias=bf_sb[:, m:m + 1], scale=1.0)
                    nc.vector.tensor_mul(psl["fl"], psl["fl"], cl_ap)
                    nc.vector.tensor_add(c_t, c_t, psl["fl"])
                    nc.scalar.activation(out=psl["fr"], in_=psl["fr"],
                                         func=AF.Sigmoid,
                                         bias=bf_sb[:, m:m + 1], scale=1.0)
                    nc.vector.tensor_mul(psl["fr"], psl["fr"], cr_ap)
                    nc.vector.tensor_add(c_dst, c_t, psl["fr"])
                else:
                    nc.vector.tensor_copy(c_dst, c_t)
                o_sb = evac.tile(shape, F32, tag="gsb", bufs=2, name="o_sb")
                nc.scalar.activation(out=o_sb, in_=psl["o"], func=AF.Sigmoid,
                                     bias=biou_sb[:, MF + m:MF + m + 1],
                                     scale=1.0)
                th = ps.tile(shape, F32, tag="th", bufs=2, name="th")
                nc.scalar.activation(out=th, in_=c_dst, func=AF.Tanh)
                if root:
                    h_t = evac.tile(shape, F32, tag="hroot", name="h_t")
                    nc.vector.tensor_mul(h_t, o_sb, th)
                    nc.sync.dma_start(out=outh.ap()[m], in_=h_t[:, :, 0])
                    return
                nc.vector.tensor_mul(h_dst, o_sb, th)
                if hc_dst is not None:
                    nc.sync.dma_start(out=hc_dst, in_=hc_t)

            def hc_scratch(n, name):
                # chunk-major: [m, col_chunk, h|c, part, tree, col] so every
                # DMA slice collapses to <=3 AP dims (t,c merge)
                W = min(n, 128)
                return dram.tile([MF, max(n // 128, 1), 2, 128, T, W], F16,
                                 tag="hc", name=name)

            def emit_leaf(wh_interleave=False):
                _mark(nc, "leaf")
                hcA = hc_scratch(NL, "hcA")
                ncc = 128
                for ci in range(NL // ncc):
                    cs = slice(ci * ncc, (ci + 1) * ncc)
                    # alternate between the (leaf-idle) hl/hr tags for
                    # double buffering at zero extra SBUF cost
                    x_t = stream.tile([128, KX, T, ncc], F16,
                                      tag=("hl" if ci % 2 == 0 else "hr"),
                                      name="x_t")
                    load_h_tile(x_t, xT.ap()[:, :, :, cs], KX)
                    for m in range(MF):
                        if wh_interleave and m % 2 == 1:
                            # spread the resident-weight load in 1.3 MiB
                            # pieces so x/wx never queue behind a large blob
                            load_wh_chunk(ci * 4 + m // 2, 1)
                        wxg = stream.tile([128, 3, KX, 128], F16, tag="wxc",
                                          bufs=2, name="wxg")
                        nc.sync.dma_start(out=wxg, in_=wxT.ap()[m])
                        psl = {}
                        for g, gt in enumerate(("i", "o", "u")):
                            pt = ps.tile([128, T, ncc], F32, tag=gt,
                                         bufs=(2 if gt == "o" else 1),
                                         name=f"ps_{gt}")
                            for k in range(KX):
                                nc.tensor.matmul(pt, lhsT=wxg[:, g, k],
                                                 rhs=x_t[:, k],
                                                 start=(k == 0),
                                                 stop=(k == KX - 1))
                            psl[gt] = pt
                        per_m_outputs(
                            psl, m, None, None,
                            hcA[m, ci].rearrange("s p t c -> p s t c"),
                            None, None, leaf=True)
                return hcA

            def emit_merged_level(n, ht_prev, ct_prev, root):
                """Tail level (n<=16 output nodes/tree): all MF chunks in one
                psum tile per gate; bias pre-filled, merged elementwise."""
                shape = [128, MF, T, n]
                ht_cur = ct_cur = None
                if not root:
                    ht_cur = evac.tile(shape, F16, tag="ht", bufs=2,
                                       name="ht_cur")
                    ct_cur = evac.tile(shape, F16, tag="ct", bufs=2,
                                       name="ct_cur")
                psl = {}
                for gt in ("i", "o", "u", "fl", "fr"):
                    psl[gt] = ps.tile(shape, F32, tag=gt,
                                      bufs=(2 if gt == "o" else 1),
                                      name=f"ps_{gt}")
                for g, gt in enumerate(("i", "o", "u", "fl", "fr")):
                    pt = psl[gt]
                    for m in range(MF):
                        # start=True zeroes only the written slice on HW, so
                        # per-m chains can share one bank; bias added after
                        for k in range(KH):
                            kk, lr = k % (KH // 2), k // (KH // 2)
                            nc.tensor.matmul(
                                pt[:, m],
                                lhsT=wh_sb[:, k, g * D + m * 128:
                                           g * D + (m + 1) * 128],
                                rhs=ht_prev[:, kk, :, lr * n:(lr + 1) * n],
                                start=(k == 0), stop=(k == KH - 1),
                                skip_group_check=True)
                    nc.vector.tensor_add(pt, pt,
                                         bias_cols(gt).to_broadcast(shape))
                i_sb = evac.tile(shape, F32, tag="gsb", bufs=2, name="i_sb")
                nc.scalar.activation(out=i_sb, in_=psl["i"], func=AF.Sigmoid)
                nc.scalar.activation(out=psl["u"], in_=psl["u"], func=AF.Tanh)
                c_t = evac.tile(shape, F32, tag="c", bufs=1, name="c_t")
                nc.vector.tensor_mul(c_t, i_sb, psl["u"])
                nc.scalar.activation(out=psl["fl"], in_=psl["fl"],
                                     func=AF.Sigmoid)
                nc.vector.tensor_mul(psl["fl"], psl["fl"],
                                     ct_prev[:, :, :, 0:n])
                nc.vector.tensor_add(c_t, c_t, psl["fl"])
                nc.scalar.activation(out=psl["fr"], in_=psl["fr"],
                                     func=AF.Sigmoid)
                nc.vector.tensor_mul(psl["fr"], psl["fr"],
                                     ct_prev[:, :, :, n:2 * n])
                o_sb = evac.tile(shape, F32, tag="gsb", bufs=2, name="o_sb")
                nc.scalar.activation(out=o_sb, in_=psl["o"], func=AF.Sigmoid)
                th = ps.tile(shape, F32, tag="th", bufs=2, name="th")
                if root:
                    c_f = evac.tile(shape, F32, tag="croot", name="c_f")
                    nc.vector.tensor_add(c_f, c_t, psl["fr"])
                    nc.scalar.activation(out=th, in_=c_f, func=AF.Tanh)
                    h_t = evac.tile(shape, F32, tag="hroot", name="h_t")
                    nc.vector.tensor_mul(h_t, o_sb, th)
                    nc.sync.dma_start(
                        out=outh.ap().rearrange("m p t -> p m t"),
                        in_=h_t[:, :, :, 0])
                    return None, None
                nc.vector.tensor_add(ct_cur, c_t, psl["fr"])
                nc.scalar.activation(out=th, in_=ct_cur, func=AF.Tanh)
                nc.vector.tensor_mul(ht_cur, o_sb, th)
                return ht_cur, ct_cur

            def emit_internal(hcA):
                n = NL
                ht_prev = ct_prev = None        # SBUF tail state
                while n > 1:
                    n //= 2                     # output nodes per tree
                    if n < TAIL_STOP:
                        break
                    _mark(nc, f"level_n{n}")
                    root = n == 1
                    if n <= N_MERGED:
                        ht_prev, ct_prev = emit_merged_level(
                            n, ht_prev, ct_prev, root)
                        continue
                    sbuf_out = n <= N_SBUF_TAIL and not root
                    ht_cur = ct_cur = hcB = None
                    if root:
                        pass
                    elif sbuf_out:
                        ht_cur = evac.tile([128, MF, T, n], F16, tag="ht",
                                           bufs=2, name="ht_cur")
                        ct_cur = evac.tile([128, MF, T, n], F16, tag="ct",
                                           bufs=2, name="ct_cur")
                    else:
                        hcB = hc_scratch(n, "hcB")
                    ncc = min(n, 128)
                    small_in = 2 * n <= 128     # children fit in one chunk
                    dj = max(n // 128, 1)       # chunk distance left->right
                    for ci in range(n // ncc):
                        if small_in:
                            # children cols [0, 2n) in chunk 0: one DMA for
                            # both halves; slice left/right in SBUF
                            hl_t = stream.tile([128, KH // 2, T, 2 * n], F16,
                                               tag="hl", name="hl_t")
                            load_h_tile(hl_t,
                                        hcA[:, 0, 0].rearrange(
                                            "m p t c -> p m t c"), KH // 2)
                            hr_t = None
                        else:
                            hl_t = stream.tile([128, KH // 2, T, ncc], F16,
                                               tag="hl", name="hl_t")
                            load_h_tile(hl_t,
                                        hcA[:, ci, 0].rearrange(
                                            "m p t c -> p m t c"), KH // 2)
                            hr_t = stream.tile([128, KH // 2, T, ncc], F16,
                                               tag="hr", name="hr_t")
                            load_h_tile(hr_t,
                                        hcA[:, ci + dj, 0].rearrange(
                                            "m p t c -> p m t c"), KH // 2)
                        for m in range(MF):
                            if small_in:
                                cin = stream.tile([128, T, 2 * n], F16,
                                                  tag="wxc", bufs=2,
                                                  name="cin")
                                nc.sync.dma_start(out=cin, in_=hcA[m, 0, 1])
                                cl_ap = cin[:, :, 0:n]
                                cr_ap = cin[:, :, n:2 * n]
                            else:
                                cin = stream.tile([128, 2, T, ncc], F16,
                                                  tag="wxc", bufs=2,
                                                  name="cin")
                                nc.sync.dma_start(
                                    out=cin,
                                    in_=hcA[m, :, 1].rearrange(
                                        "(two g) p t c -> p two g t c",
                                        two=2)[:, :, ci])
                                cl_ap = cin[:, 0]
                                cr_ap = cin[:, 1]
                            psl = {}
                            for g, gt in enumerate(
                                    ("i", "o", "u", "fl", "fr")):
                                pt = ps.tile([128, T, ncc], F32, tag=gt,
                                             bufs=(2 if gt == "o" else 1),
                                             name=f"ps_{gt}")
                                for k in range(KH):
                                    kk = k % (KH // 2)
                                    if small_in:
                                        lr = k // (KH // 2)
                                        rhs = hl_t[:, kk, :,
                                                   lr * n:(lr + 1) * n]
                                    else:
                                        rhs = (hl_t[:, kk] if k < KH // 2
                                               else hr_t[:, kk])
                                    nc.tensor.matmul(
                                        pt,
                                        lhsT=wh_sb[:, k,
                                                   g * D + m * 128:
                                                   g * D + (m + 1) * 128],
                                        rhs=rhs,
                                        start=(k == 0), stop=(k == KH - 1))
                                psl[gt] = pt
                            if root:
                                per_m_outputs(psl, m, cl_ap, cr_ap,
                                              None, None, None,
                                              leaf=False, root=True)
                            elif sbuf_out:
                                per_m_outputs(psl, m, cl_ap, cr_ap,
                                              None, ht_cur[:, m], ct_cur[:, m],
                                              leaf=False)
                            else:
                                per_m_outputs(
                                    psl, m, cl_ap, cr_ap,
                                    hcB[m, ci].rearrange(
                                        "s p t c -> p s t c"),
                                    None, None, leaf=False)
                    if sbuf_out:
                        ht_prev, ct_prev = ht_cur, ct_cur
                    else:
                        hcA = hcB

            if reps == 1:
                hcA = emit_leaf(wh_interleave=True)
                emit_internal(hcA)
            else:
                for ci in range(4):
                    load_wh_chunk(ci, KH // 4)
                with tc.For_i(0, reps, 1):
                    hcA = emit_leaf()
                    emit_internal(hcA)

    nc.compile()
    return nc


def _get_nc(reps=1):
    key = f"nc{reps}"
    if key not in _CACHE:
        _CACHE[key] = _build_program(reps)
    return _CACHE[key]


def _prep_inputs(inputs, w_fioux, b_fioux, w_iouh, w_fh):
    """Host-side prep: permute/transpose/cast, build one in_map per core."""
    perm = _bitrev(9)
    # weights / biases are replicated across cores
    # wxT [m, p, g, k, c]: W_x[gate g, row m*128+c, col k*128+p]
    wxT = np.ascontiguousarray(
        w_fioux[D:].reshape(3, MF, 128, KX, 128).transpose(1, 4, 0, 3, 2)
    ).astype(np.float16)
    whT = np.ascontiguousarray(
        np.concatenate([w_iouh, w_fh], axis=0).T).astype(np.float16) \
        .reshape(KH, 128, 5 * D)
    biou = np.ascontiguousarray(
        b_fioux[D:].astype(np.float32).reshape(3 * MF, 128).T)
    bf = np.ascontiguousarray(
        b_fioux[:D].astype(np.float32).reshape(MF, 128).T)
    in_maps = []
    for c in range(NCORES):
        xc = inputs[c * T:(c + 1) * T][:, perm, :]        # [T, NL, D]
        xT = np.ascontiguousarray(
            xc.reshape(T, NL, KX, 128).transpose(3, 2, 0, 1)
        ).astype(np.float16)                              # [128, KX, T, NL]
        in_maps.append({"xT": xT, "wxT": wxT, "whT": whT,
                        "biou": biou, "bf": bf})
    return in_maps


def _assemble(results):
    out = np.zeros((B, D), np.float32)
    for c in range(NCORES):
        oh = results[c]["outh"].reshape(D, T)             # [feat, tree]
        out[c * T:(c + 1) * T] = oh.T
    return out


def kernel(inputs, w_fioux, b_fioux, w_iouh, w_fh):
    inputs = np.asarray(inputs, np.float32)
    w_fioux = np.asarray(w_fioux, np.float32)
    b_fioux = np.asarray(b_fioux, np.float32)
    w_iouh = np.asarray(w_iouh, np.float32)
    w_fh = np.asarray(w_fh, np.float32)
    nc = _get_nc()
    in_maps = _prep_inputs(inputs, w_fioux, b_fioux, w_iouh, w_fh)
    res = run_bass_kernel_spmd(nc, in_maps, core_ids=list(range(NCORES)))
    return _assemble(res.results)


# ---------------------------------------------------------------------------
# benchmarking helper (not used by the grader): builds the jitted SPMD
# callable once so repeated executions can be timed without re-lowering.
def _bench(inputs, w_fioux, b_fioux, w_iouh, w_fh, iters=20, reps=1):
    import time

    import jax
    from jax.experimental.shard_map import shard_map
    from jax.sharding import Mesh, PartitionSpec

    from concourse import bass2jax

    nc = _get_nc(reps)
    in_maps = _prep_inputs(np.asarray(inputs, np.float32),
                           np.asarray(w_fioux, np.float32),
                           np.asarray(b_fioux, np.float32),
                           np.asarray(w_iouh, np.float32),
                           np.asarray(w_fh, np.float32))
    bass2jax.install_neuronx_cc_hook()

    partition_name = (nc.partition_id_tensor.name
                      if nc.partition_id_tensor else None)
    in_names, out_names, out_avals, zero_outs = [], [], [], []
    for alloc in nc.m.functions[0].allocations:
        if not isinstance(alloc, mybir.MemoryLocationSet):
            continue
        name = alloc.memorylocations[0].name
        if alloc.kind == "ExternalInput":
            if name != partition_name:
                in_names.append(name)
        elif alloc.kind == "ExternalOutput":
            out_names.append(name)
            shape = tuple(alloc.tensor_shape)
            dtype = mybir.dt.np(alloc.dtype)
            out_avals.append(jax.core.ShapedArray(shape, dtype))
            zero_outs.append(np.zeros(shape, dtype))
    n_params = len(in_names)
    n_outs = len(out_avals)
    all_in_names = in_names + out_names + \
        ([partition_name] if partition_name else [])

    def _body(*args):
        operands = list(args)
        if partition_name is not None:
            operands.append(bass2jax.partition_id_tensor())
        outs = bass2jax._bass_exec_p.bind(
            *operands,
            out_avals=tuple(out_avals),
            in_names=tuple(all_in_names),
            out_names=tuple(out_names),
            lowering_input_output_aliases=(),
            sim_require_finite=True,
            sim_require_nnan=True,
            nc=nc,
        )
        return tuple(outs)

    devices = jax.devices()[:NCORES]
    mesh = Mesh(np.asarray(devices), ("core",))
    in_specs = (PartitionSpec("core"),) * (n_params + n_outs)
    out_specs = (PartitionSpec("core"),) * n_outs
    donate = tuple(range(n_params, n_params + n_outs))
    sharded = jax.jit(
        shard_map(_body, mesh=mesh, in_specs=in_specs, out_specs=out_specs,
                  check_rep=False),
        donate_argnums=donate, keep_unused=True)

    concat_in = [np.concatenate([in_maps[c][nm] for c in range(NCORES)],
                                axis=0)
                 for nm in in_names]
    dev_in = [jax.device_put(
        a, jax.sharding.NamedSharding(mesh, PartitionSpec("core")))
        for a in concat_in]

    def one_run():
        zeros = [jax.device_put(
            np.zeros((NCORES * z.shape[0], *z.shape[1:]), z.dtype),
            jax.sharding.NamedSharding(mesh, PartitionSpec("core")))
            for z in zero_outs]
        t0 = time.perf_counter()
        out = sharded(*dev_in, *zeros)
        jax.block_until_ready(out)
        return time.perf_counter() - t0, out

    one_run()                                    # compile warm-up
    times = []
    out = None
    for _ in range(iters):
        dt, out = one_run()
        times.append(dt)
    out_np = [np.asarray(o) for o in out]
    results = [{nm: out_np[i].reshape(NCORES, *out_avals[i].shape)[c]
                for i, nm in enumerate(out_names)}
               for c in range(NCORES)]
    return _assemble(results), times
